# revision 1
# baseline (speedup 1.0000x reference)
"""AttentionBlock Trainium2 kernel (v3).

Reference computation (B=16, C=512, H=W=32, n_heads=4, d_k=128):
    xs   = x.reshape(B,C,S).T            # [B, S, C],  S = 1024
    qkv  = xs @ w_proj.T + b_proj        # [B, S, 1536]
    S_   = einsum('bihd,bjhd->bijh', q, k) * d_k**-0.5
    attn = softmax(S_, axis=1)           # over the QUERY axis i (source quirk)
    res  = einsum('bijh,bjhd->bihd', attn, v)
    out  = res @ w_out.T + b_out + xs    # residual
    return out.T.reshape(B, C, H, W)

Data-parallel over batch, 2 batches per core on 8 cores.

HW cost model (measured via microbenchmarks on this setup; ldw-opt is
disabled so every matmul pays a serial LdWeights):
    matmul ~= 50ns + 0.417ns * (ldw_cols * dtype_bytes + out_cols)
    fp8 [128,128]x[128,512]:               270 ns
    fp8 DoubleRow [128,2,128]x[128,2,512]: 370 ns (256-wide contraction)
    f32r [128,128]x[128,512]:             ~476 ns (4-byte ldw!)
    ACT Exp [128,1024] + accum:           1362 ns
    DVE psum->fp8 [128,512]:              ~898 ns; psum->f32/bf16 ~700 ns
So ALL matmuls run fp8: scores non-DR (contraction is d_k=128), the
projections and AV in DoubleRow (256-wide contraction halves instruction
count). Back-to-back accumulation into the same psum bank stalls (~+300ns),
so chunk pairs alternate psum banks A/B.

Engine budget per core (2 batches): PE ~105us (the wall), ACT 87us,
DVE ~73us, Pool ~22us. Emitted as a 64-step software pipeline (one exp per
step) with projection/output units statically placed to meet dataflow
deadlines.

Numerics (numpy sim of this quantization chain: rel err ~1.3e-2 vs the 2e-2
gate; HW has measured below sim): exp(s*scale - 3) keeps e in fp8 range;
v8 = (v * inv_den) * 1024; resT8 = racc/64; wo8 = 512*w_out.T;
out = psum/8192 + (x + b_out)  [b_out folded into the f32 residual input
on the host].
"""
import sys

for _p in (
    "/opt/trn_rl_repo",
    "/root/.axon_site",
    "/root/.axon_site/_ro/trn_rl_repo",
    "/root/.axon_site/_ro/pypackages",
):
    if _p not in sys.path:
        sys.path.append(_p)

import numpy as np

B = 16
C = 512
S = 1024  # H*W
NH = 4
DK = 128
F = NH * DK  # 512
NCORES = 8
BL = B // NCORES  # batches per core
KT = C // 128  # 4  contraction tiles over channels
ST = S // 128  # 8  seq tiles
NT = S // 512  # 2  free-dim chunks of 512
SCALE = float(DK) ** -0.5
ESHIFT = -3.0   # exp(s*SCALE + ESHIFT): keeps fp8 e <= ~45
VSC = 1024.0    # v8 = (v * inv_den) * VSC
RSC = 1.0 / 64.0   # resT8 = racc * RSC -> 16*res
WOSC = 512.0    # wo8 = fp8(WOSC * w_out.T)
OSC = 1.0 / (VSC * RSC * WOSC)  # out = psum * OSC + (x + b_out)

_CACHE: dict = {}


def _build(repeat=1):
    """Build the kernel. repeat>1 wraps the per-call workload in an on-device
    For_i loop (timing only: amortizes the ~10ms axon dispatch)."""
    import contextlib

    import concourse.bass as bass
    import concourse.tile as tile
    from concourse import bacc, mybir

    F32 = mybir.dt.float32
    BF16 = mybir.dt.bfloat16
    FP8 = mybir.dt.float8e4
    U8 = mybir.dt.uint8
    EXP = mybir.ActivationFunctionType.Exp
    DR = mybir.MatmulPerfMode.DoubleRow
    ADD = mybir.AluOpType.add
    MULT = mybir.AluOpType.mult
    ts = bass.ts

    nc = bacc.Bacc("TRN2", debug=False)
    # xb = x + b_out (host-folded); x8/w*8 are e4m3 bytes shipped as uint8
    xb_d = nc.dram_tensor("xb", [BL, C, S], F32, kind="ExternalInput").ap()
    x8_d = nc.dram_tensor("x8", [BL, C, S], U8, kind="ExternalInput").ap()
    wqk_d = nc.dram_tensor("wqk8", [C, 2 * F], U8, kind="ExternalInput").ap()
    wv_d = nc.dram_tensor("wv8", [C, F], U8, kind="ExternalInput").ap()
    wo_d = nc.dram_tensor("wo8", [F, C], U8, kind="ExternalInput").ap()
    bias_d = nc.dram_tensor("bias", [128, 2 * NH + F + 1], F32, kind="ExternalInput").ap()
    out_d = nc.dram_tensor("out", [BL, C, S], F32, kind="ExternalOutput").ap()

    x8r = x8_d.bitcast(FP8)
    wqk_r = wqk_d.rearrange("(k p) m -> p k m", p=128).bitcast(FP8)
    wv_r = wv_d.rearrange("(k p) m -> p k m", p=128).bitcast(FP8)
    wo_r = wo_d.rearrange("(k p) m -> p k m", p=128).bitcast(FP8)

    with tile.TileContext(nc) as tc:
        with (
            tc.tile_pool(name="const", bufs=1) as constp,
            tc.tile_pool(name="xp", bufs=2) as xp,
            tc.tile_pool(name="qkp", bufs=2) as qkp,
            tc.tile_pool(name="vp", bufs=2) as vp,
            tc.tile_pool(name="rp", bufs=2) as rp,
            tc.tile_pool(name="ep", bufs=4) as epool,
            tc.tile_pool(name="v8p", bufs=6) as v8pool,
            tc.tile_pool(name="small", bufs=8) as smallp,
            tc.tile_pool(name="otp", bufs=5) as otp,
            # psum: pp = [128,512]x2 proj banks (A/B alternation);
            # ps = [128,1024]x2 score tiles; pr = [128,1024]x1 AV accumulator.
            tc.tile_pool(name="pp", bufs=2, space="PSUM") as pp,
            tc.tile_pool(name="ps", bufs=2, space="PSUM") as ps,
            tc.tile_pool(name="pr", bufs=1, space="PSUM") as pr,
        ):
            wqk_sb = constp.tile([128, KT, 2 * F], FP8)
            wv_sb = constp.tile([128, KT, F], FP8)
            wo_sb = constp.tile([128, KT, C], FP8)
            bias_sb = constp.tile([128, 2 * NH + F + 1], F32)
            xb_sbs = [xp.tile([128, KT, S], F32, name=f"xb{b}", tag="xb") for b in range(BL)]
            x8_sbs = [xp.tile([128, KT, S], FP8, name=f"x8_{b}", tag="x8") for b in range(BL)]

            # DMA order: bias first, then b0's fp8 x + qk weights (gate the
            # prologue), wv + b1 fp8 x, wo, the f32 residual xb last.
            nc.sync.dma_start(out=bias_sb, in_=bias_d)
            nc.sync.dma_start(out=x8_sbs[0], in_=x8r[0].rearrange("(k p) m -> p k m", p=128))
            nc.sync.dma_start(out=wqk_sb, in_=wqk_r)
            nc.sync.dma_start(out=wv_sb, in_=wv_r)
            nc.sync.dma_start(out=x8_sbs[1], in_=x8r[1].rearrange("(k p) m -> p k m", p=128))
            nc.sync.dma_start(out=wo_sb, in_=wo_r)
            for b in range(BL):
                nc.sync.dma_start(
                    out=xb_sbs[b], in_=xb_d[b].rearrange("(k p) m -> p k m", p=128)
                )

            b_qk = bias_sb[:, 0 : 2 * NH]
            b_v = bias_sb[:, 2 * NH : 2 * NH + F]
            b_shift = bias_sb[:, 2 * NH + F :]

            rep_ctx = tc.For_i(0, repeat, 1) if repeat > 1 else contextlib.nullcontext()
            with rep_ctx:
                _body(
                    nc, xb_sbs, x8_sbs, qkp, vp, rp, epool, v8pool, smallp, otp,
                    pp, ps, pr, wqk_sb, wv_sb, wo_sb, b_qk, b_v, b_shift, out_d,
                    F32, BF16, FP8, EXP, DR, ADD, MULT, ts,
                )

    nc.compile()
    return nc


def _body(
    nc, xb_sbs, x8_sbs, qkp, vp, rp, epool, v8pool, smallp, otp,
    pp, ps, pr, wqk_sb, wv_sb, wo_sb, b_qk, b_v, b_shift, out_d,
    F32, BF16, FP8, EXP, DR, ADD, MULT, ts,
):
    qk_sb = [qkp.tile([128, 2 * NH, S], FP8, name=f"qksb{b}", tag="qksb") for b in range(BL)]
    v_sb = [vp.tile([128, ST, F], BF16, name=f"vsb{b}", tag="vsb") for b in range(BL)]
    resT8 = [rp.tile([128, NH, S], FP8, name=f"resT{b}", tag="resT") for b in range(BL)]

    # ---- filler units: 4 DoubleRow matmuls each, A/B bank alternation ----
    def qk_unit(b, t):
        # q/k f-tile t, split into two step-parts so same-bank psum
        # accumulations are >=3 instructions apart (avoids the RMW stall)
        cell = {}

        def part(kp):
            def emit():
                if kp == 0:
                    cell["A"] = pp.tile([128, 512], F32, name="pA", tag="pp")
                    cell["B"] = pp.tile([128, 512], F32, name="pB", tag="pp")
                w_pair = wqk_sb[:, 2 * kp : 2 * kp + 2, ts(t, 128)]
                for n, acc in ((0, cell["A"]), (1, cell["B"])):
                    nc.tensor.matmul(
                        acc, w_pair, x8_sbs[b][:, 2 * kp : 2 * kp + 2, ts(n, 512)],
                        start=(kp == 0), stop=(kp == KT // 2 - 1), perf_mode=DR,
                    )
                if kp == KT // 2 - 1:
                    for n, acc in ((0, cell["A"]), (1, cell["B"])):
                        nc.vector.tensor_scalar_add(
                            qk_sb[b][:, t, ts(n, 512)], acc, b_qk[:, t : t + 1]
                        )
            return emit

        return part(0), part(1)

    def v_unit(b, stp):
        # V rows for s-tiles (2*stp, 2*stp+1), two step-parts (see qk_unit)
        cell = {}

        def part(kp):
            def emit():
                if kp == 0:
                    cell["A"] = pp.tile([128, 512], F32, name="pA", tag="pp")
                    cell["B"] = pp.tile([128, 512], F32, name="pB", tag="pp")
                kpair = slice(2 * kp, 2 * kp + 2)
                for st, acc in ((2 * stp, cell["A"]), (2 * stp + 1, cell["B"])):
                    nc.tensor.matmul(
                        acc, x8_sbs[b][:, kpair, ts(st, 128)], wv_sb[:, kpair, :],
                        start=(kp == 0), stop=(kp == KT // 2 - 1), perf_mode=DR,
                    )
                if kp == KT // 2 - 1:
                    for st, acc in ((2 * stp, cell["A"]), (2 * stp + 1, cell["B"])):
                        nc.vector.tensor_add(v_sb[b][:, st, :], acc, b_v)
            return emit

        return part(0), part(1)

    def out_unit(b, ct):
        # out c-tile ct (two step-parts): psum = wo8.T @ resT8; out = psum*OSC + xb
        cell = {}

        def part(hp):
            def emit():
                if hp == 0:
                    cell["A"] = pp.tile([128, 512], F32, name="pA", tag="pp")
                    cell["B"] = pp.tile([128, 512], F32, name="pB", tag="pp")
                wo_pair = wo_sb[:, 2 * hp : 2 * hp + 2, ts(ct, 128)]
                for n, acc in ((0, cell["A"]), (1, cell["B"])):
                    nc.tensor.matmul(
                        acc, wo_pair, resT8[b][:, 2 * hp : 2 * hp + 2, ts(n, 512)],
                        start=(hp == 0), stop=(hp == NH // 2 - 1), perf_mode=DR,
                    )
                if hp == NH // 2 - 1:
                    for n, acc in ((0, cell["A"]), (1, cell["B"])):
                        ot = otp.tile([128, 512], F32, name="ot", tag="ot")
                        nc.vector.scalar_tensor_tensor(
                            ot, acc, OSC, xb_sbs[b][:, ct, ts(n, 512)], MULT, ADD,
                        )
                        nc.sync.dma_start(
                            out=out_d[b, ts(ct, 128), ts(n, 512)], in_=ot
                        )
            return emit

        return part(0), part(1)

    # ---- partial / final output projection for the last batch ----
    # b1's out-proj is split: heads 0-1 accumulate into part_sb during b1-h3's
    # attention; after the last drain only heads 2-3 + one add remain.
    part_sb = otp.tile([128, KT, S], F32, name="part", tag="part", bufs=1)

    def part_unit(ct, n):
        def emit():
            acc = pp.tile([128, 512], F32, name="pA", tag="pp")
            nc.tensor.matmul(
                acc, wo_sb[:, 0:2, ts(ct, 128)], resT8[1][:, 0:2, ts(n, 512)],
                start=True, stop=True, perf_mode=DR,
            )
            nc.vector.scalar_tensor_tensor(
                part_sb[:, ct, ts(n, 512)], acc, OSC,
                xb_sbs[1][:, ct, ts(n, 512)], MULT, ADD,
            )
        return emit

    def tail_unit(ct):
        def emit():
            ot = otp.tile([128, 1024], F32, name="ot2", tag="ot2")
            for n in range(NT):
                acc = pp.tile([128, 512], F32, name="pA", tag="pp")
                nc.tensor.matmul(
                    acc, wo_sb[:, 2:4, ts(ct, 128)], resT8[1][:, 2:4, ts(n, 512)],
                    start=True, stop=True, perf_mode=DR,
                )
                nc.vector.scalar_tensor_tensor(
                    ot[:, ts(n, 512)], acc, OSC, part_sb[:, ct, ts(n, 512)], MULT, ADD,
                )
            eng = nc.sync if ct % 2 == 0 else nc.scalar
            eng.dma_start(out=out_d[1, ts(ct, 128), :], in_=ot)
        return emit

    # ---- static filler schedule (dataflow deadlines in comments) ----
    fillers: dict = {}

    def put(s, *ems):
        fillers.setdefault(s, []).extend(ems)

    def put2(s, unit):
        p1, p2 = unit
        put(s, p1)
        put(s + 1, p2)

    put2(0, v_unit(0, 1))    # fin s1; v_sc(st2) emitted s6 (b0 lag 4)
    put2(2, qk_unit(0, 2))   # h1 scores read at s8
    put2(4, qk_unit(0, 3))
    put2(6, v_unit(0, 2))    # fin s7 <= v_sc(st4)@s8
    put2(8, v_unit(0, 3))    # fin s9 <= v_sc(st6)@s10
    put2(10, qk_unit(0, 4))  # h2 scores s16
    put2(12, qk_unit(0, 5))
    put2(14, qk_unit(0, 6))  # h3 scores s24
    put2(16, qk_unit(0, 7))
    put2(18, qk_unit(1, 0))  # b1 h0 scores s32
    put2(20, qk_unit(1, 1))
    put2(22, v_unit(1, 0))   # v_sc(b1,st0)@s34 (b1 lag 2)
    put2(24, v_unit(1, 1))
    put2(26, v_unit(1, 2))
    put2(28, v_unit(1, 3))
    put2(30, qk_unit(1, 2))  # b1 h1 scores s40
    put2(32, qk_unit(1, 3))
    put2(34, qk_unit(1, 4))  # b1 h2 scores s48
    put2(36, qk_unit(1, 5))
    put2(38, qk_unit(1, 6))  # b1 h3 scores s56
    put2(40, qk_unit(1, 7))
    put2(43, out_unit(0, 0))  # resT8(b0) complete ~s36 (b0 lag 4)
    put2(45, out_unit(0, 1))
    put2(47, out_unit(0, 2))
    put2(49, out_unit(0, 3))
    for i, (ct, n) in enumerate([(c, n) for c in range(KT) for n in range(NT)]):
        put(52 + i, part_unit(ct, n))  # resT8(b1,h0/h1) drained by ~s50

    # ---- the 64-step pipeline ----
    # b0 runs with a deeper vsc/AV lag (4) so the early v_proj units have
    # slack; b1 uses lag 2 to shorten the tail. The AV pair trails its v_sc
    # by one extra step (Pool latency) except on the final head.
    steps = [(b, h, jt) for b in range(BL) for h in range(NH) for jt in range(ST)]
    LAG = {0: 4, 1: 2}
    AVLAG = {0: 5, 1: 3}
    pairs: dict = {}
    raccs: dict = {}
    ssums: dict = {}

    def emit_av(b, h, jtp):
        e8p, v8p = pairs.pop((b, h, jtp))
        if jtp == 0:
            raccs[(b, h)] = pr.tile([128, S], F32, name="racc", tag="racc")
        racc = raccs[(b, h)]
        for n in range(NT):
            nc.tensor.matmul(
                racc[:, ts(n, 512)], v8p, e8p[:, :, ts(n, 512)],
                start=(jtp == 0), stop=(jtp == ST // 2 - 1), perf_mode=DR,
            )
        if jtp == ST // 2 - 1:
            for n in range(NT):
                nc.vector.tensor_scalar_mul(
                    resT8[b][:, h, ts(n, 512)], racc[:, ts(n, 512)], RSC
                )

    def emit_vsc(tgt):
        if tgt < 0 or tgt >= len(steps):
            return
        tb, th, tjt = steps[tgt]
        e8p, v8p = pairs[(tb, th, tjt // 2)]
        ssum = ssums.pop(tgt)
        nc.vector.reciprocal(ssum[:, 1:2], ssum[:, 0:1])
        nc.gpsimd.tensor_scalar(
            v8p[:, tjt % 2, :], v_sb[tb][:, tjt, ts(th, DK)],
            ssum[:, 1:2], VSC, MULT, MULT,
        )

    def emit_deferred(s):
        # per-step deferred work: vsc for step s-LAG; AV pair at s-AVLAG
        # (one step later than its v_sc, hiding the Pool latency), except
        # the final head which uses the short lag to keep the tail tight.
        for tb in (0, 1):
            tgt = s - LAG[tb]
            if 0 <= tgt < len(steps) and steps[tgt][0] == tb:
                emit_vsc(tgt)
                _, th, tjt = steps[tgt]
                if tb == 1 and th == NH - 1 and tjt % 2 == 1:
                    emit_av(tb, th, tjt // 2)  # last head: av right after vsc
            tgt2 = s - AVLAG[tb]
            if 0 <= tgt2 < len(steps) and steps[tgt2][0] == tb:
                _, th2, tjt2 = steps[tgt2]
                if (tb, th2) == (1, NH - 1):
                    continue
                if tjt2 % 2 == 1:
                    emit_av(tb, th2, tjt2 // 2)

    # prologue: first head's q/k tiles + first v pair
    for p1, p2 in (qk_unit(0, 1), qk_unit(0, 0), v_unit(0, 0)):
        p1()
        p2()

    for s, (b, h, jt) in enumerate(steps):
        sacc = ps.tile([128, S], F32, name="sacc", tag="sacc")
        for n in range(NT):
            nc.tensor.matmul(
                sacc[:, ts(n, 512)],
                qk_sb[b][:, 2 * h + 1, ts(jt, 128)],
                qk_sb[b][:, 2 * h, ts(n, 512)],
                start=True, stop=True,
            )
        jtp, parity = jt // 2, jt % 2
        if parity == 0:
            pairs[(b, h, jtp)] = (
                epool.tile([128, 2, S], FP8, name="e8", tag="e8"),
                v8pool.tile([128, 2, DK], FP8, name="v8", tag="v8"),
            )
        e8p, v8p = pairs[(b, h, jtp)]
        ssum = smallp.tile([128, 2], F32, name="ssum", tag="ssum")
        ssums[s] = ssum
        nc.scalar.activation(
            out=e8p[:, parity, :], in_=sacc, func=EXP,
            scale=SCALE, bias=b_shift, accum_out=ssum[:, 0:1],
        )
        emit_deferred(s)
        for f in fillers.get(s, []):
            f()

    for s in range(len(steps), len(steps) + 5):
        emit_deferred(s)

    # tail: last batch's remaining output projection (heads 2-3 + add)
    for ct in range(KT):
        tail_unit(ct)()


def bass_ts(i, size):
    import concourse.bass as bass

    return bass.ts(i, size)


def _fp8_bytes(a):
    import ml_dtypes

    return np.ascontiguousarray(
        np.asarray(a, dtype=np.float32).astype(ml_dtypes.float8_e4m3).view(np.uint8)
    )


def _prep_inputs(x, w_proj, b_proj, w_out, b_out):
    """Host-side quantization + reshaping into the layouts the kernel expects."""
    x_f = np.ascontiguousarray(x.reshape(B, C, S), dtype=np.float32)
    xb = x_f + np.asarray(b_out, dtype=np.float32)[None, :, None]  # residual + b_out
    x8 = _fp8_bytes(x_f)
    wT = np.asarray(w_proj, dtype=np.float32).T  # [C, 3*F], f = h*384 + j
    w_qkT = np.concatenate(
        [wT[:, h * 384 : h * 384 + 256] for h in range(NH)], axis=1
    )  # [C, 2F]; col tile t=2h -> q_h, t=2h+1 -> k_h
    w_vT = np.concatenate(
        [wT[:, h * 384 + 256 : h * 384 + 384] for h in range(NH)], axis=1
    )  # [C, F]
    w_outT = WOSC * np.asarray(w_out, dtype=np.float32).T  # [F, C]
    b_proj = np.asarray(b_proj, dtype=np.float32)
    b_qk = np.stack(
        [
            b_proj[h * 384 + half * 128 : h * 384 + half * 128 + 128]
            for h in range(NH)
            for half in range(2)
        ],
        axis=1,
    )  # [128, 2*NH], col t matches qk tile order
    b_v = np.concatenate(
        [b_proj[h * 384 + 256 : h * 384 + 384] for h in range(NH)]
    )  # [F]
    b_v_bcast = np.broadcast_to(b_v, (128, F))
    shift_col = np.full((128, 1), ESHIFT, dtype=np.float32)
    bias = np.ascontiguousarray(
        np.concatenate([b_qk, b_v_bcast, shift_col], axis=1), dtype=np.float32
    )  # [128, 2*NH + F + 1]
    return xb, x8, _fp8_bytes(w_qkT), _fp8_bytes(w_vT), _fp8_bytes(w_outT), bias


def kernel(x, w_proj, b_proj, w_out, b_out, n_heads):
    from concourse.bass_utils import run_bass_kernel_spmd

    assert int(n_heads) == NH
    xb, x8, wqk8, wv8, wo8, bias = _prep_inputs(x, w_proj, b_proj, w_out, b_out)

    if "nc" not in _CACHE:
        _CACHE["nc"] = _build()
    nc = _CACHE["nc"]

    in_maps = [
        {
            "xb": np.ascontiguousarray(xb[c * BL : (c + 1) * BL]),
            "x8": np.ascontiguousarray(x8[c * BL : (c + 1) * BL]),
            "wqk8": wqk8,
            "wv8": wv8,
            "wo8": wo8,
            "bias": bias,
        }
        for c in range(NCORES)
    ]
    res = run_bass_kernel_spmd(nc, in_maps, list(range(NCORES)))
    out = np.concatenate([res.results[c]["out"] for c in range(NCORES)], axis=0)
    return out.reshape(B, C, 32, 32)

